# revision 1
# baseline (speedup 1.0000x reference)
"""Trainium2 Bass kernel for nn_ApproxCompressor (v4).

Reference (per sample n):
    alpha = sigmoid(z_alpha); h[k] = (1-alpha)*alpha^k (k<16384)
    env   = causal_conv(mean_c x^2, h); LG = log(env + 1e-5)
    quadratic-knee gain; out = gain * x.

Kernel strategy (8 cores x 4 samples, pure data parallel):
  * Merged layout: sample s owns partitions [32s, 32s+32); each partition
    holds a contiguous 4096-sample signal run.  Per-sample scalars are
    (128,1) SBUF columns (ACT bias/scale, tensor_scalar operands), so one
    instruction per stage covers all four samples (SPMD-clean).
  * The 16K-tap exponential FIR == one-pole IIR y[t] = a*y[t-1] + D[t]
    (truncation tail a^16384 underflows; asserted).  DVE tensor_tensor_scan
    runs the recurrence per partition with data0 = stride-0 broadcast of the
    alpha column (HW-validated exact); free-dim chunks chained via `initial`.
    Cross-partition carries are then fixed exactly: a block-diagonal 128x128
    decay matmul (PE) yields per-partition initial states applied to the
    first JF columns via one scalar_tensor_tensor with a host power table.
  * Quadratic knee, branch-free and cancellation-free in f32:
        A = relu(d+W); C = min(A, 2W); log_gain = c/(4W) * C * (2A - C)
  * I/O: the host ships the input already in bf16 *device layout*
    (partition-major rows, halves interleaved) so every DMA descriptor is a
    16KB-contiguous run; output is written in device layout (bf16) and
    unshuffled on the host.  DMAs are split across both HWDGE queues
    (SP + ACT) by partition halves.
"""

import os
import sys

import numpy as np


def _import_concourse():
    try:
        import concourse.bass  # noqa: F401
    except ImportError:
        for p in ("/opt/trn_rl_repo", "/root/.axon_site/_ro/trn_rl_repo"):
            if os.path.isdir(p) and p not in sys.path:
                sys.path.insert(0, p)
        import concourse.bass  # noqa: F401


_import_concourse()

import ml_dtypes  # noqa: E402
import concourse.bass as bass  # noqa: E402
import concourse.tile as tile  # noqa: E402
from concourse import bacc, mybir  # noqa: E402

N, C, L = 32, 2, 131072
NCORES = 8
NLOC = N // NCORES  # 4 samples/core
P = 128
SPP = P // NLOC  # 32 partitions/sample
FCH = L // SPP  # 4096 samples per partition row
NIN = 2  # input/output halves
W_IN = FCH // NIN  # 2048
NCH = 4  # compute chunks
W_CH = FCH // NCH  # 1024
JF = 256
EPS = 1e-5
K_FIR = 16384
ROW = C * FCH  # 8192 elems per device-layout row

F32 = mybir.dt.float32
BF16 = mybir.dt.bfloat16

PRM_ALPHA, PRM_LNSCALE, PRM_B1, PRM_W2, PRM_C4W, PRM_EPS, PRM_BM2W = 0, 1, 2, 3, 4, 5, 6
NPRM = 8
ACT_SET_ID = 6
Q_DVE_PARTS = 96  # 32-aligned

TRACE_RESULT = {}


def _bcast(col_ap, n):
    return bass.AP(col_ap.tensor, col_ap.offset, [list(col_ap.ap[0]), [0, n]])


def build_nc():
    AF = mybir.ActivationFunctionType
    OP = mybir.AluOpType

    nc = bacc.Bacc("TRN2", target_bir_lowering=False, num_devices=NCORES)
    # device-layout input/output: row p, free = h*(C*W_IN) + ch*W_IN + j
    xd_ext = nc.declare_dram_parameter("xd", [P, ROW], BF16, isOutput=False)
    prm_ext = nc.declare_dram_parameter("prm", [P, NPRM], F32, isOutput=False)
    tri_ext = nc.declare_dram_parameter("tri", [P, P], F32, isOutput=False)
    pw_ext = nc.declare_dram_parameter("pw", [P, JF], F32, isOutput=False)
    od_ext = nc.declare_dram_parameter("od", [P, ROW], BF16, isOutput=True)

    HB = C * W_IN  # 4096: elems per half-row

    with tile.TileContext(nc) as tc:
        atl = mybir.InstLoadActFuncSet(
            name=nc.get_next_instruction_name(), ins=[], outs=[],
            act_func_set_id=ACT_SET_ID,
        )
        nc.scalar.add_instruction(atl)
        with (
            tc.tile_pool(name="pc", bufs=1) as pc,
            tc.tile_pool(name="pin", bufs=NIN + 1) as pin,
            tc.tile_pool(name="po", bufs=NIN + 1) as po,
            tc.tile_pool(name="py", bufs=NCH + 1) as py,
            tc.tile_pool(name="pk", bufs=4) as pk,
            tc.tile_pool(name="pps", bufs=2, space=bass.MemorySpace.PSUM) as pps,
        ):
            prm = pc.tile([P, NPRM], F32, tag="prm")
            nc.gpsimd.dma_start(out=prm[:], in_=prm_ext[:])
            tri = pc.tile([P, P], F32, tag="tri")
            nc.gpsimd.dma_start(out=tri[:], in_=tri_ext[:])
            pw = pc.tile([P, JF], F32, tag="pw")
            nc.gpsimd.dma_start(out=pw[:], in_=pw_ext[:])

            a_col = prm[:, PRM_ALPHA : PRM_ALPHA + 1]
            lnscale_col = prm[:, PRM_LNSCALE : PRM_LNSCALE + 1]
            b1_col = prm[:, PRM_B1 : PRM_B1 + 1]
            w2_col = prm[:, PRM_W2 : PRM_W2 + 1]
            c4w_col = prm[:, PRM_C4W : PRM_C4W + 1]
            eps_col = prm[:, PRM_EPS : PRM_EPS + 1]
            bm2w_col = prm[:, PRM_BM2W : PRM_BM2W + 1]

            # ---- input: per-half tiles, 2 calls each (partition split) -----
            xht = []
            for h in range(NIN):
                xh = pin.tile([P, HB], BF16, tag="xh")
                nc.sync.dma_start(
                    out=xh[0:64, :], in_=xd_ext[0:64, h * HB : (h + 1) * HB]
                )
                nc.scalar.dma_start(
                    out=xh[64:128, :], in_=xd_ext[64:128, h * HB : (h + 1) * HB]
                )
                xht.append(xh)

            def xsl(ch, k):
                h, r = k // 2, k % 2
                return xht[h][:, ch * W_IN + r * W_CH : ch * W_IN + r * W_CH + W_CH]

            # ---- energy + chained scans per chunk --------------------------
            y1h = []
            for _h in range(NIN):
                y1t = py.tile([P, W_IN], F32, tag="y1")
                y1h.append(y1t)
            for k in range(NCH):
                h, r = k // 2, k % 2
                # square both channels in one ACT op via a 3D strided view
                x2 = xsl(0, k)
                x2v = bass.AP(x2.tensor, x2.offset,
                              [list(x2.ap[0]), [W_IN, C], [1, W_CH]])
                sq = pk.tile([P, C * W_CH], BF16, tag="sq")
                sqv = sq[:].rearrange("p (c t) -> p c t", c=C)
                nc.scalar.activation(sqv, x2v, AF.Square)
                D = pk.tile([P, W_CH], BF16, tag="D")
                nc.vector.tensor_tensor(
                    D[:], sq[:, 0:W_CH], sq[:, W_CH : 2 * W_CH], OP.add
                )

                ysl = y1h[h][:, r * W_CH : (r + 1) * W_CH]
                if k == 0:
                    init = 0.0
                elif k % 2 == 1:
                    init = y1h[h][:, r * W_CH - 1 : r * W_CH]
                else:
                    init = y1h[h - 1][:, W_IN - 1 : W_IN]
                nc.vector.tensor_tensor_scan(
                    ysl, _bcast(a_col, W_CH), D[:], init, OP.mult, OP.add
                )

            # ---- cross-partition carry fix ---------------------------------
            s_col = pps.tile([P, 1], F32, tag="s_col")
            nc.tensor.matmul(
                s_col[:], tri[:], y1h[NIN - 1][:, W_IN - 1 : W_IN],
                start=True, stop=True,
            )
            nc.vector.scalar_tensor_tensor(
                y1h[0][:, 0:JF], pw[:], s_col[:, 0:1], y1h[0][:, 0:JF],
                OP.mult, OP.add,
            )

            # ---- knee + gain apply; chunk 0 last (waits on the fix) --------
            ods = []
            for _h in range(NIN):
                odt = po.tile([P, HB], BF16, tag="od")
                ods.append(odt)
            for k in [1, 0, 2, 3][:NCH]:
                h, r = k // 2, k % 2
                y1sl = y1h[h][:, r * W_CH : (r + 1) * W_CH]
                LG = pk.tile([P, W_CH], F32, tag="LG")
                nc.scalar.activation(
                    LG[:], y1sl, AF.Ln, bias=eps_col, scale=lnscale_col
                )
                A = pk.tile([P, W_CH], F32, tag="A")
                nc.scalar.activation(A[:], LG[:], AF.Relu, bias=b1_col)
                Ct = pk.tile([P, W_CH], F32, tag="LG")
                nc.vector.tensor_scalar_min(Ct[:], A[:], w2_col)
                Z = pk.tile([P, W_CH], F32, tag="A")
                nc.vector.scalar_tensor_tensor(
                    Z[:], A[:], 2.0, Ct[:], OP.mult, OP.subtract
                )
                Q = pk.tile([P, W_CH], F32, tag="Q")
                nc.vector.tensor_tensor(Q[:], Ct[:], Z[:], OP.mult)
                gain = pk.tile([P, W_CH], BF16, tag="gain")
                nc.scalar.activation(gain[:], Q[:], AF.Exp, scale=c4w_col)

                od = ods[h]
                o0 = od[:, r * W_CH : r * W_CH + W_CH]
                ov3 = bass.AP(o0.tensor, o0.offset,
                              [list(o0.ap[0]), [W_IN, C], [1, W_CH]])
                x2 = xsl(0, k)
                xv3 = bass.AP(x2.tensor, x2.offset,
                              [list(x2.ap[0]), [W_IN, C], [1, W_CH]])
                g0 = gain[:]
                gv3 = bass.AP(g0.tensor, g0.offset,
                              [list(g0.ap[0]), [0, C], [1, W_CH]])
                nc.vector.tensor_tensor(ov3, gv3, xv3, OP.mult)
                dof = h * HB + r * W_CH
                dsl = [slice(dof, dof + W_CH),
                       slice(dof + W_IN, dof + W_IN + W_CH)]
                nc.sync.dma_start(
                    out=od_ext[0:64, dsl[0]], in_=od[0:64, r * W_CH : r * W_CH + W_CH]
                )
                nc.sync.dma_start(
                    out=od_ext[0:64, dsl[1]], in_=od[0:64, W_IN + r * W_CH : W_IN + r * W_CH + W_CH]
                )
                nc.gpsimd.dma_start(
                    out=od_ext[64:128, dsl[0]], in_=od[64:128, r * W_CH : r * W_CH + W_CH]
                )
                nc.gpsimd.dma_start(
                    out=od_ext[64:128, dsl[1]], in_=od[64:128, W_IN + r * W_CH : W_IN + r * W_CH + W_CH]
                )


    nc.finalize()
    return nc


def host_params(z_alpha, log_threshold, log_ratio, log_knee):
    z = z_alpha.astype(np.float64).reshape(-1)
    alpha = 1.0 / (1.0 + np.exp(-z))
    aK = np.exp(K_FIR * np.log(alpha))
    assert np.all(aK < 1e-6), "FIR tail non-negligible; needs shift correction"
    aJ = np.exp(JF * np.log(alpha))
    assert np.all(aJ < 1e-7), "carry-fix reach JF too small for this alpha"
    T = log_threshold.astype(np.float64).reshape(-1) - 6.0
    R = 1.0 + np.exp(log_ratio.astype(np.float64).reshape(-1))
    W = np.exp(log_knee.astype(np.float64).reshape(-1))
    c = 1.0 / R - 1.0

    n = alpha.shape[0]
    prms, tris, pws = [], [], []
    j = np.arange(1, JF + 1, dtype=np.float64)
    kq = np.arange(SPP)[None, :] - 1 - np.arange(SPP)[:, None]
    for c0 in range(n // NLOC):
        sl = slice(c0 * NLOC, (c0 + 1) * NLOC)
        a4, T4, W4, c4 = alpha[sl], T[sl], W[sl], c[sl]
        prm = np.zeros((P, NPRM), np.float64)
        rep = np.repeat
        prm[:, PRM_ALPHA] = rep(a4, SPP)
        prm[:, PRM_LNSCALE] = rep(0.5 * (1.0 - a4), SPP)
        prm[:, PRM_B1] = rep(W4 - T4, SPP)
        prm[:, PRM_W2] = rep(2.0 * W4, SPP)
        prm[:, PRM_C4W] = rep(c4 / (4.0 * W4), SPP)
        prm[:, PRM_EPS] = EPS
        prm[:, PRM_BM2W] = rep(-2.0 * W4, SPP)
        prms.append(prm.astype(np.float32))

        tri = np.zeros((P, P), np.float64)
        pwm = np.zeros((P, JF), np.float64)
        for s in range(NLOC):
            expo = FCH * kq * np.log(a4[s])
            m = (kq >= 0) & (expo > -100.0)
            blk = np.zeros((SPP, SPP))
            blk[m] = np.exp(expo[m])
            tri[s * SPP : (s + 1) * SPP, s * SPP : (s + 1) * SPP] = blk
            pwm[s * SPP : (s + 1) * SPP, :] = np.exp(j * np.log(a4[s]))[None, :]
        tris.append(tri.astype(np.float32))
        pws.append(pwm.astype(np.float32))
    return prms, tris, pws


def shuffle_in(x_core):
    """(NLOC, C, L) f32 -> (P, ROW) bf16 device layout."""
    xb = x_core.astype(np.float32).astype(ml_dtypes.bfloat16)
    v = xb.reshape(NLOC, C, SPP, NIN, W_IN).transpose(0, 2, 3, 1, 4)
    return np.ascontiguousarray(v.reshape(P, ROW))


def unshuffle_out(od):
    """(P, ROW) bf16 device layout -> (NLOC, C, L) f32."""
    v = od.reshape(NLOC, SPP, NIN, C, W_IN).astype(np.float32)
    return v.transpose(0, 3, 1, 2, 4).reshape(NLOC, C, L)


def _ensure_ntff_hook():
    import types

    try:
        from antenv.axon_hooks import get_axon_ntff_profile_hook  # noqa: F401

        return
    except ImportError:
        pass
    try:
        from trn_agent_boot.trn_boot import _ntff_profile_via_ctypes
    except ImportError:
        return
    hook = _ntff_profile_via_ctypes("/opt/axon/libaxon_pjrt.so")
    mod = types.ModuleType("antenv.axon_hooks")
    mod._hook = hook
    mod.get_axon_ntff_profile_hook = lambda: mod._hook

    def set_axon_ntff_profile_hook(h):
        mod._hook = h

    mod.set_axon_ntff_profile_hook = set_axon_ntff_profile_hook
    import antenv

    sys.modules["antenv.axon_hooks"] = mod
    antenv.axon_hooks = mod


def kernel(input_signals, z_alpha, log_threshold, log_ratio, log_knee):
    from concourse.bass_utils import run_bass_kernel_spmd

    x = np.asarray(input_signals, np.float32)
    prms, tris, pws = host_params(
        np.asarray(z_alpha), np.asarray(log_threshold),
        np.asarray(log_ratio), np.asarray(log_knee),
    )

    nc = build_nc()
    core_ids = list(range(NCORES))
    in_maps = [
        {
            "xd": shuffle_in(x[i * NLOC : (i + 1) * NLOC]),
            "prm": prms[i],
            "tri": tris[i],
            "pw": pws[i],
        }
        for i in core_ids
    ]

    trace = os.environ.get("BASS_KERNEL_TRACE", "0") == "1"
    if trace:
        _ensure_ntff_hook()
    res = run_bass_kernel_spmd(nc, in_maps, core_ids, trace=trace)
    if trace:
        TRACE_RESULT["exec_time_ns"] = res.exec_time_ns
        TRACE_RESULT["results"] = res

    out = np.empty((N, C, L), np.float32)
    for i in core_ids:
        out[i * NLOC : (i + 1) * NLOC] = unshuffle_out(
            np.asarray(res.results[i]["od"])
        )
    return out

